# revision 8
# baseline (speedup 1.0000x reference)
"""Sliding-window (radius-8, K=17) single-head attention along W.

Full problem: feature/position [2, 128, 64, 256] f32; 1x1 convs Wq/Wk (+bias)
produce q/k; scores over a 17-wide window along W; softmax (zero-padded
windows contribute exp(0)=1 to the denominator); output is the attn-weighted
sum of windows of x = feature + position.

Sharding: data-parallel over (B, H) — the 128 (b, h) rows are independent;
each of the 8 cores gets 16 rows. Per row (x_row = [C=128, W=256]):
  q = (Wq/sqrt(C)) x + bq/sqrt(C);  k = Wk x + bk          (PE matmuls + bias)
  S[w, w'] = q^T k + bandmask  (full 256x256; mask pre-written to PSUM via a
             PE copy-matmul, scores accumulate on top)
  att = exp(S) bf16; denom = rowsum(att) + n_oob; att *= 1/denom
  out = x @ att^T   (PE transposes of att and x, then 2 accumulating matmuls)

Matmul path runs in bf16 (fast weight load + 1 cyc/row); scores accumulate in
fp32 PSUM and the exp input stays fp32. The softmax skips max-subtraction:
scores are O(10) here, well within exp/fp32 range; out-of-band entries get
-1e9 from the mask. Zero-padded (out-of-range) window positions are accounted
by adding their exact count (n_oob, exp(0)=1 each) to the denominator.
Elementwise work is spread over ACT/DVE/GPSIMD to keep all engines busy.
"""

import numpy as np
from contextlib import ExitStack

import concourse.bass as bass
import concourse.bacc as bacc
import concourse.mybir as mybir
import concourse.tile as tile
from concourse.bass_utils import run_bass_kernel_spmd

B, C, H, W = 2, 128, 64, 256
R = 8
NCORES = 8
ROWS = B * H // NCORES        # 16 (b, h) rows per core
CORES_PER_B = NCORES // B     # 4
F32 = mybir.dt.float32
BF = mybir.dt.bfloat16
EXP = mybir.ActivationFunctionType.Exp
NEG = -1e9
RL = 4                        # rows per input DMA
RS = 2                        # rows per output DMA


def build_nc():
    nc = bacc.Bacc(trn_type="TRN2")
    f_ext = nc.dram_tensor("feature", [C, ROWS, W], F32, kind="ExternalInput")
    p_ext = nc.dram_tensor("position", [C, ROWS, W], F32, kind="ExternalInput")
    wq_ext = nc.dram_tensor("wqt", [C, C], BF, kind="ExternalInput")
    wk_ext = nc.dram_tensor("wkt", [C, C], BF, kind="ExternalInput")
    id_ext = nc.dram_tensor("ident", [C, C], BF, kind="ExternalInput")
    bq_ext = nc.dram_tensor("bqv", [C, 1], F32, kind="ExternalInput")
    bk_ext = nc.dram_tensor("bkv", [C, 1], F32, kind="ExternalInput")
    mask_ext = nc.dram_tensor("bandmask", [C, 2 * W], BF, kind="ExternalInput")
    oob_ext = nc.dram_tensor("oob", [C, 2], F32, kind="ExternalInput")
    out_ext = nc.dram_tensor("out", [C, ROWS, W], F32, kind="ExternalOutput")

    with tile.TileContext(nc) as tc, ExitStack() as ctx:
        const = ctx.enter_context(tc.tile_pool(name="const", bufs=1))
        wq_t = const.tile([C, C], BF)
        nc.sync.dma_start(wq_t[:], wq_ext[:])
        wk_t = const.tile([C, C], BF)
        nc.sync.dma_start(wk_t[:], wk_ext[:])
        bq_t = const.tile([C, 1], F32)
        nc.sync.dma_start(bq_t[:], bq_ext[:])
        bk_t = const.tile([C, 1], F32)
        nc.sync.dma_start(bk_t[:], bk_ext[:])
        mask_t = const.tile([C, 2 * W], BF)
        nc.sync.dma_start(mask_t[:], mask_ext[:])
        oob_t = const.tile([C, 2], F32)
        nc.sync.dma_start(oob_t[:], oob_ext[:])
        ident = const.tile([C, C], BF)
        nc.sync.dma_start(ident[:], id_ext[:])

        inp = ctx.enter_context(tc.tile_pool(name="inp", bufs=2))
        xp = ctx.enter_context(tc.tile_pool(name="x", bufs=3))
        qkp = ctx.enter_context(tc.tile_pool(name="qk", bufs=3))
        attp = ctx.enter_context(tc.tile_pool(name="att", bufs=3))
        smallp = ctx.enter_context(tc.tile_pool(name="small", bufs=4))
        sbT = ctx.enter_context(tc.tile_pool(name="sbT", bufs=3))
        ps = ctx.enter_context(tc.tile_pool(name="ps", bufs=2, space="PSUM"))

        ft = pt = o_ps = None
        for r in range(ROWS):
            if r % RL == 0:
                ft = inp.tile([C, RL, W], F32, tag="ft")
                nc.sync.dma_start(ft[:], f_ext[:, r : r + RL, :])
                pt = inp.tile([C, RL, W], F32, tag="pt")
                nc.sync.dma_start(pt[:], p_ext[:, r : r + RL, :])
            j = r % RL

            xt = xp.tile([C, W], BF)
            nc.gpsimd.tensor_add(xt[:], ft[:, j, :], pt[:, j, :])

            qk_ps = ps.tile([C, 2 * W], F32, tag="qk")
            nc.tensor.matmul(qk_ps[:, 0:W], wq_t[:], xt[:], start=True, stop=True)
            nc.tensor.matmul(qk_ps[:, W : 2 * W], wk_t[:], xt[:], start=True, stop=True)

            qk_sb = qkp.tile([C, 2 * W], BF)
            nc.scalar.add(qk_sb[:, 0:W], qk_ps[:, 0:W], bq_t[:])
            nc.vector.tensor_scalar_add(
                qk_sb[:, W : 2 * W], qk_ps[:, W : 2 * W], bk_t[:]
            )

            # band mask lands in PSUM via a PE copy (ident.T @ mask); the two
            # score matmuls then accumulate q^T k on top of it.
            s_ps = ps.tile([C, 2 * W], F32, tag="s")
            nc.tensor.matmul(s_ps[:], ident[:], mask_t[:], start=True, stop=False)
            nc.tensor.matmul(
                s_ps[:, 0:W],
                qk_sb[:, 0:128],
                qk_sb[:, W : 2 * W],
                start=False,
                stop=False,
            )
            nc.tensor.matmul(
                s_ps[:, W : 2 * W],
                qk_sb[:, 128:256],
                qk_sb[:, W : 2 * W],
                start=False,
                stop=True,
            )

            att = attp.tile([C, 2 * W], BF)
            nc.scalar.activation(att[:], s_ps[:], EXP)

            den = smallp.tile([C, 2], F32, tag="den")
            nc.vector.tensor_reduce(
                den[:],
                att[:].rearrange("c (t w) -> c t w", t=2),
                axis=mybir.AxisListType.X,
                op=mybir.AluOpType.add,
            )
            rden = smallp.tile([C, 2], F32, tag="rden")
            nc.vector.tensor_add(rden[:], den[:], oob_t[:])
            nc.vector.reciprocal(rden[:], rden[:])

            nc.gpsimd.tensor_scalar_mul(att[:, 0:W], att[:, 0:W], rden[:, 0:1])
            nc.gpsimd.tensor_scalar_mul(
                att[:, W : 2 * W], att[:, W : 2 * W], rden[:, 1:2]
            )

            # attT = [C0 | C1]: C0 rows are keys w' 0:128, C1 rows keys 128:256;
            # columns are queries w 0:256.
            at_ps = ps.tile([C, 2 * W], BF, tag="tstage")
            nc.tensor.transpose(at_ps[:, 0:128], att[:, 0:128], ident[:])
            nc.tensor.transpose(at_ps[:, 128:256], att[:, 256:384], ident[:])
            nc.tensor.transpose(at_ps[:, 256:384], att[:, 128:256], ident[:])
            nc.tensor.transpose(at_ps[:, 384:512], att[:, 384:512], ident[:])

            xt_ps = ps.tile([C, W], BF, tag="qk")
            nc.tensor.transpose(xt_ps[:, 0:128], xt[:, 0:128], ident[:])
            nc.tensor.transpose(xt_ps[:, 128:256], xt[:, 128:256], ident[:])

            aT = sbT.tile([C, 2 * W], BF, tag="aT")
            nc.scalar.copy(aT[:], at_ps[:])
            xT = sbT.tile([C, W], BF, tag="xT")
            nc.vector.tensor_copy(xT[:], xt_ps[:])

            if r % RS == 0:
                o_ps = ps.tile([C, RS * W], F32, tag="out")
            os_ = o_ps[:, (r % RS) * W : (r % RS + 1) * W]
            nc.tensor.matmul(os_, xT[:, 0:128], aT[:, 0:W], start=True, stop=False)
            nc.tensor.matmul(
                os_, xT[:, 128:256], aT[:, W : 2 * W], start=False, stop=True
            )
            if r % RS == RS - 1:
                o_sb = sbT.tile([C, RS * W], F32, tag="osb")
                nc.any.tensor_copy(o_sb[:], o_ps[:])
                nc.sync.dma_start(out_ext[:, r - RS + 1 : r + 1, :], o_sb[:])

    nc.compile()
    return nc


def host_consts(Wq, bq, Wk, bk):
    import ml_dtypes

    sc = 1.0 / np.sqrt(np.float32(C))
    wqt = np.ascontiguousarray(Wq.astype(np.float32).T * sc).astype(ml_dtypes.bfloat16)
    bqv = np.ascontiguousarray((bq.astype(np.float32) * sc).reshape(C, 1))
    wkt = np.ascontiguousarray(Wk.astype(np.float32).T).astype(ml_dtypes.bfloat16)
    bkv = np.ascontiguousarray(bk.astype(np.float32).reshape(C, 1))

    ident = np.eye(C, dtype=np.float32).astype(ml_dtypes.bfloat16)
    bandmask = np.full((C, 2 * W), NEG, dtype=np.float32)
    oob = np.zeros((C, 2), dtype=np.float32)
    for t in range(2):
        for p in range(C):
            w = t * 128 + p
            lo, hi = max(0, w - R), min(W, w + R + 1)
            bandmask[p, t * W + lo : t * W + hi] = 0.0
            oob[p, t] = max(0, R - w) + max(0, w - (W - 1 - R))
    bandmask = bandmask.astype(ml_dtypes.bfloat16)
    return wqt, bqv, wkt, bkv, bandmask, oob, ident


def core_inputs(feature, position, Wq, bq, Wk, bk):
    wqt, bqv, wkt, bkv, bandmask, oob, ident = host_consts(Wq, bq, Wk, bk)
    in_maps = []
    for i in range(NCORES):
        b = i // CORES_PER_B
        h0 = (i % CORES_PER_B) * ROWS
        in_maps.append(
            {
                "feature": np.ascontiguousarray(
                    feature[b, :, h0 : h0 + ROWS, :], dtype=np.float32
                ),
                "position": np.ascontiguousarray(
                    position[b, :, h0 : h0 + ROWS, :], dtype=np.float32
                ),
                "wqt": wqt,
                "ident": ident,
                "wkt": wkt,
                "bqv": bqv,
                "bkv": bkv,
                "bandmask": bandmask,
                "oob": oob,
            }
        )
    return in_maps


def kernel(feature, position, Wq, bq, Wk, bk):
    in_maps = core_inputs(feature, position, Wq, bq, Wk, bk)
    nc = build_nc()
    res = run_bass_kernel_spmd(nc, in_maps, list(range(NCORES)))
    out = np.empty((B, C, H, W), dtype=np.float32)
    for i in range(NCORES):
        b = i // CORES_PER_B
        h0 = (i % CORES_PER_B) * ROWS
        out[b, :, h0 : h0 + ROWS, :] = res.results[i]["out"]
    return out


# revision 9
# speedup vs baseline: 2.1977x; 2.1977x over previous
"""Sliding-window (radius-8, K=17) single-head attention along W.

Full problem: feature/position [2, 128, 64, 256] f32; 1x1 convs Wq/Wk (+bias)
produce q/k; scores over a 17-wide window along W; softmax (zero-padded
windows contribute exp(0)=1 to the denominator); output is the attn-weighted
sum of windows of x = feature + position.

Sharding: data-parallel over (B, H) — the 128 (b, h) rows are independent;
each of the 8 cores gets 16 rows, processed two rows per iteration to
amortize per-instruction overhead. Per row (x_row = [C=128, W=256]):
  q = (Wq/sqrt(C)) x + bq/sqrt(C);  k = Wk x + bk          (PE matmuls + bias)
  S[w, w'] = q^T k + bandmask  (full 256x256; the mask is pre-written into
             PSUM by a PE copy-matmul and scores accumulate on top)
  att = exp(S) bf16; denom = rowsum(att) + n_oob; att *= 1/denom
  out = x @ att^T   (PE transposes of att and x, then accumulating matmuls)

The matmul path runs in bf16 (fast weight load + 1 cyc/row); scores
accumulate in fp32 PSUM and the exp input stays fp32. Softmax skips
max-subtraction: scores are O(10) here, well inside exp/fp32 range;
out-of-band entries carry -1e9 from the mask. Zero-padded (out-of-range)
window positions are accounted by adding their exact count (n_oob,
exp(0)=1 each) to the denominator before the reciprocal.
"""

import numpy as np
from contextlib import ExitStack

import concourse.bass as bass
import concourse.bacc as bacc
import concourse.mybir as mybir
import concourse.tile as tile
from concourse.bass_utils import run_bass_kernel_spmd

B, C, H, W = 2, 128, 64, 256
R = 8
NCORES = 8
ROWS = B * H // NCORES        # 16 (b, h) rows per core
CORES_PER_B = NCORES // B     # 4
F32 = mybir.dt.float32
BF = mybir.dt.bfloat16
EXP = mybir.ActivationFunctionType.Exp
NEG = -1e9
RL = 4                        # rows per input DMA


def build_nc():
    nc = bacc.Bacc(trn_type="TRN2")
    f_ext = nc.dram_tensor("feature", [C, ROWS, W], F32, kind="ExternalInput")
    p_ext = nc.dram_tensor("position", [C, ROWS, W], F32, kind="ExternalInput")
    wq_ext = nc.dram_tensor("wqt", [C, C], BF, kind="ExternalInput")
    wk_ext = nc.dram_tensor("wkt", [C, C], BF, kind="ExternalInput")
    id_ext = nc.dram_tensor("ident", [C, C], BF, kind="ExternalInput")
    bq_ext = nc.dram_tensor("bqv", [C, 1], F32, kind="ExternalInput")
    bk_ext = nc.dram_tensor("bkv", [C, 1], F32, kind="ExternalInput")
    mask_ext = nc.dram_tensor("bandmask", [C, 2 * W], BF, kind="ExternalInput")
    oob_ext = nc.dram_tensor("oob4", [C, 4, 1], F32, kind="ExternalInput")
    out_ext = nc.dram_tensor("out", [C, ROWS, W], F32, kind="ExternalOutput")

    with tile.TileContext(nc) as tc, ExitStack() as ctx:
        const = ctx.enter_context(tc.tile_pool(name="const", bufs=1))
        wq_t = const.tile([C, C], BF)
        nc.sync.dma_start(wq_t[:], wq_ext[:])
        wk_t = const.tile([C, C], BF)
        nc.sync.dma_start(wk_t[:], wk_ext[:])
        bq_t = const.tile([C, 1], F32)
        nc.sync.dma_start(bq_t[:], bq_ext[:])
        bk_t = const.tile([C, 1], F32)
        nc.sync.dma_start(bk_t[:], bk_ext[:])
        mask_t = const.tile([C, 2 * W], BF)
        nc.sync.dma_start(mask_t[:], mask_ext[:])
        oob_t = const.tile([C, 4, 1], F32)
        nc.sync.dma_start(oob_t[:], oob_ext[:])
        ident = const.tile([C, C], BF)
        nc.sync.dma_start(ident[:], id_ext[:])

        inp = ctx.enter_context(tc.tile_pool(name="inp", bufs=2))
        xp = ctx.enter_context(tc.tile_pool(name="x", bufs=3))
        qkp = ctx.enter_context(tc.tile_pool(name="qk", bufs=3))
        attp = ctx.enter_context(tc.tile_pool(name="att", bufs=3))
        smallp = ctx.enter_context(tc.tile_pool(name="small", bufs=4))
        sbT = ctx.enter_context(tc.tile_pool(name="sbT", bufs=3))
        ps1 = ctx.enter_context(tc.tile_pool(name="ps1", bufs=1, space="PSUM"))
        ps2 = ctx.enter_context(tc.tile_pool(name="ps2", bufs=2, space="PSUM"))

        ft = pt = None
        for it in range(ROWS // 2):
            r = 2 * it
            if r % RL == 0:
                ft = inp.tile([C, RL, W], F32, tag="ft")
                nc.sync.dma_start(ft[:], f_ext[:, r : r + RL, :])
                pt = inp.tile([C, RL, W], F32, tag="pt")
                nc.sync.dma_start(pt[:], p_ext[:, r : r + RL, :])
            j = r % RL

            # x for both rows: [C, 2, W] bf16
            xt = xp.tile([C, 2, W], BF)
            nc.gpsimd.tensor_add(xt[:], ft[:, j : j + 2, :], pt[:, j : j + 2, :])
            xv = xt[:].rearrange("c r w -> c (r w)")

            # q|k for both rows: [C, 1024] fp32 PSUM (2 banks)
            qk_ps = ps1.tile([C, 4 * W], F32, tag="qk")
            nc.tensor.matmul(qk_ps[:, 0 : 2 * W], wq_t[:], xv, start=True, stop=True)
            nc.tensor.matmul(
                qk_ps[:, 2 * W : 4 * W], wk_t[:], xv, start=True, stop=True
            )

            qk_sb = qkp.tile([C, 4 * W], BF)
            nc.scalar.add(qk_sb[:, 0 : 2 * W], qk_ps[:, 0 : 2 * W], bq_t[:])
            nc.vector.tensor_scalar_add(
                qk_sb[:, 2 * W : 4 * W], qk_ps[:, 2 * W : 4 * W], bk_t[:]
            )

            # scores for both rows: bank per row; mask lands via PE copy-matmul
            # (ident.T @ mask), the two per-row score matmuls accumulate on top.
            s_ps = ps1.tile([C, 4 * W], F32, tag="s")
            for rr in range(2):
                o0 = rr * 2 * W
                q0 = rr * 2 * 128
                k0 = 2 * W + rr * W
                nc.tensor.matmul(
                    s_ps[:, o0 : o0 + 2 * W], ident[:], mask_t[:],
                    start=True, stop=False,
                )
                nc.tensor.matmul(
                    s_ps[:, o0 : o0 + W],
                    qk_sb[:, q0 : q0 + 128],
                    qk_sb[:, k0 : k0 + W],
                    start=False, stop=False,
                )
                nc.tensor.matmul(
                    s_ps[:, o0 + W : o0 + 2 * W],
                    qk_sb[:, q0 + 128 : q0 + 256],
                    qk_sb[:, k0 : k0 + W],
                    start=False, stop=True,
                )

            att = attp.tile([C, 4 * W], BF)
            nc.scalar.activation(att[:], s_ps[:], EXP)

            den = smallp.tile([C, 4, 1], F32, tag="den")
            nc.vector.tensor_reduce(
                den[:].rearrange("c t u -> c (t u)"),
                att[:].rearrange("c (t w) -> c t w", t=4),
                axis=mybir.AxisListType.X,
                op=mybir.AluOpType.add,
            )
            rden = smallp.tile([C, 4, 1], F32, tag="rden")
            nc.vector.scalar_tensor_tensor(
                rden[:], den[:], 1.0, oob_t[:],
                op0=mybir.AluOpType.mult, op1=mybir.AluOpType.add,
            )
            nc.vector.reciprocal(rden[:], rden[:])

            # normalize all four tiles in one broadcast multiply
            nc.vector.tensor_mul(
                att[:].rearrange("c (t w) -> c t w", t=4),
                att[:].rearrange("c (t w) -> c t w", t=4),
                rden[:].broadcast_to([C, 4, W]),
            )

            # attT per row: [C0 | C1] (keys 0:128 then 128:256, queries on free)
            at_ps = ps2.tile([C, 4 * W], BF, tag="tstage")
            for rr in range(2):
                a0 = rr * 2 * W
                nc.tensor.transpose(
                    at_ps[:, a0 : a0 + 128], att[:, a0 : a0 + 128], ident[:]
                )
                nc.tensor.transpose(
                    at_ps[:, a0 + 128 : a0 + 256],
                    att[:, a0 + W : a0 + W + 128],
                    ident[:],
                )
                nc.tensor.transpose(
                    at_ps[:, a0 + 256 : a0 + 384],
                    att[:, a0 + 128 : a0 + W],
                    ident[:],
                )
                nc.tensor.transpose(
                    at_ps[:, a0 + 384 : a0 + 512],
                    att[:, a0 + W + 128 : a0 + 2 * W],
                    ident[:],
                )

            xt_ps = ps2.tile([C, 2 * W], BF, tag="tstage")
            for rr in range(2):
                nc.tensor.transpose(
                    xt_ps[:, rr * W : rr * W + 128],
                    xt[:, rr, 0:128],
                    ident[:],
                )
                nc.tensor.transpose(
                    xt_ps[:, rr * W + 128 : (rr + 1) * W],
                    xt[:, rr, 128:256],
                    ident[:],
                )

            aT = sbT.tile([C, 4 * W], BF, tag="aT")
            nc.scalar.copy(aT[:], at_ps[:])
            xT = sbT.tile([C, 2 * W], BF, tag="xT")
            nc.vector.tensor_copy(xT[:], xt_ps[:])

            o_ps = ps2.tile([C, 2 * W], F32, tag="out")
            for rr in range(2):
                os_ = o_ps[:, rr * W : (rr + 1) * W]
                nc.tensor.matmul(
                    os_,
                    xT[:, rr * W : rr * W + 128],
                    aT[:, rr * 2 * W : rr * 2 * W + W],
                    start=True, stop=False,
                )
                nc.tensor.matmul(
                    os_,
                    xT[:, rr * W + 128 : (rr + 1) * W],
                    aT[:, rr * 2 * W + W : (rr + 1) * 2 * W],
                    start=False, stop=True,
                )
            o_sb = sbT.tile([C, 2 * W], F32, tag="osb")
            nc.any.tensor_copy(o_sb[:], o_ps[:])
            nc.sync.dma_start(out_ext[:, r : r + 2, :], o_sb[:])

    nc.compile()
    return nc


def host_consts(Wq, bq, Wk, bk):
    import ml_dtypes

    sc = 1.0 / np.sqrt(np.float32(C))
    wqt = np.ascontiguousarray(Wq.astype(np.float32).T * sc).astype(ml_dtypes.bfloat16)
    bqv = np.ascontiguousarray((bq.astype(np.float32) * sc).reshape(C, 1))
    wkt = np.ascontiguousarray(Wk.astype(np.float32).T).astype(ml_dtypes.bfloat16)
    bkv = np.ascontiguousarray(bk.astype(np.float32).reshape(C, 1))

    ident = np.eye(C, dtype=np.float32).astype(ml_dtypes.bfloat16)
    bandmask = np.full((C, 2 * W), NEG, dtype=np.float32)
    oob = np.zeros((C, 2), dtype=np.float32)
    for t in range(2):
        for p in range(C):
            w = t * 128 + p
            lo, hi = max(0, w - R), min(W, w + R + 1)
            bandmask[p, t * W + lo : t * W + hi] = 0.0
            oob[p, t] = max(0, R - w) + max(0, w - (W - 1 - R))
    bandmask = bandmask.astype(ml_dtypes.bfloat16)
    oob4 = np.concatenate([oob, oob], axis=1).reshape(C, 4, 1)
    return wqt, bqv, wkt, bkv, bandmask, oob4, ident


def core_inputs(feature, position, Wq, bq, Wk, bk):
    wqt, bqv, wkt, bkv, bandmask, oob4, ident = host_consts(Wq, bq, Wk, bk)
    in_maps = []
    for i in range(NCORES):
        b = i // CORES_PER_B
        h0 = (i % CORES_PER_B) * ROWS
        in_maps.append(
            {
                "feature": np.ascontiguousarray(
                    feature[b, :, h0 : h0 + ROWS, :], dtype=np.float32
                ),
                "position": np.ascontiguousarray(
                    position[b, :, h0 : h0 + ROWS, :], dtype=np.float32
                ),
                "wqt": wqt,
                "ident": ident,
                "wkt": wkt,
                "bqv": bqv,
                "bkv": bkv,
                "bandmask": bandmask,
                "oob4": oob4,
            }
        )
    return in_maps


def kernel(feature, position, Wq, bq, Wk, bk):
    in_maps = core_inputs(feature, position, Wq, bq, Wk, bk)
    nc = build_nc()
    res = run_bass_kernel_spmd(nc, in_maps, list(range(NCORES)))
    out = np.empty((B, C, H, W), dtype=np.float32)
    for i in range(NCORES):
        b = i // CORES_PER_B
        h0 = (i % CORES_PER_B) * ROWS
        out[b, :, h0 : h0 + ROWS, :] = res.results[i]["out"]
    return out
